# revision 2
# baseline (speedup 1.0000x reference)
"""Trainium2 Bass kernel for nn_Decoder_481036337511 — v2.

out[b,t,d] = sum_k dic[t,k] x[b,k,d];  dic = normalized real dictionary
[T=1024, 1+4*4096] built on-device from rr/theta; x is [4, 16385, 2048].

Strategy (8 cores data-parallel on D, 256 cols each):
  - Parity trick: dic blocks [ones, A, S*A, B, S*B] with S=diag((-1)^t);
    out[even] = A@U + B@W + ones-term, out[odd] = A@V + B@Z with
    U=x1+x2, V=x1-x2, W=x3+x4, Z=x3-x4 (combines on DVE/Pool, fp16).
  - Truncation: poles sorted by r DESC on host (contraction order is
    free). After unit-normalization a pole's column decays like r^t, so
    chunk c of 128 sorted poles only needs t < tcut_c with
    tcut = ln(1/delta)/ln(1/rmax); only the first chunk needs full T.
    Exact tail error computed host-side is ~1e-5 for delta=1e-3.
  - Orientation: dict chunks are the matmul STATIONARY ([128 poles,
    <=128 j] with parity-interleaved stride-2 cols), x-combines are the
    MOVING ([128, 4b x 128d] = 512 cols per LDWEIGHTS). PSUM out is
    [j, b, d]; the ones-column term is folded in as a 1-partition
    stationary matmul (0.015625 * 2 = 1/sqrt(1024)).
  - Dict built on-device in t-interleaved layout: r^t via
    tensor_tensor_scan (geometric recurrence, initial = 1/G from the
    closed-form norms), angles via iota*theta/2pi with the fp32
    big-constant round trick, sin/cos via ACT Sin table. Build work is
    batched over uniform-tcut chunk groups with stride-0 broadcast APs.
  - x shipped fp16 in matmul-ready layout [chunk, pole, blk, b, d]
    (halves HBM traffic; host does layout/cast only, no arithmetic).
    Output written as fp16 [dh, par, jb, j, b, d] tiles, host casts to
    fp32 and interleaves parities.
"""

import numpy as np
from contextlib import ExitStack

import concourse.bass as bass
import concourse.bacc as bacc
import concourse.mybir as mybir
from concourse import tile
from concourse import bass_utils

F32 = mybir.dt.float32
F16 = mybir.dt.float16
I32 = mybir.dt.int32
AF = mybir.ActivationFunctionType
OP = mybir.AluOpType

N_CORES = 8
PI = float(np.pi)
TWO_PI = float(2 * np.pi)
RSQRT_MAGIC = 0x5F3759DF
RND_BIG = 12582912.0  # 2^23 + 2^22: (q + BIG) - BIG == round(q), |q| < 2^22
NPOLE = 4096
KC = 32
B = 4
DSH = 256
T = 1024
DELTA = 1e-3  # truncation tail target (achieved tail err ~1e-5)
JGRAN = 64
L_RETAIN = 4  # max long chunks whose combines stay resident


def build_kernel_nc(jcuts, newton=1):
    """Emit the per-core Bass program for the given per-chunk j-cuts."""
    jcuts = list(jcuts)
    tcuts = [2 * j for j in jcuts]
    offs = np.concatenate([[0], np.cumsum(tcuts)]).astype(int)
    SUMT = int(offs[-1])
    maxjb = max(int(np.ceil(j / 128)) for j in jcuts)
    njb = 4  # T/256: output always covers 4 j-blocks
    DH = DSH // 128

    # build blocks: runs of equal tcut, split so Ng*tcut <= 1024
    blocks = []
    c = 0
    while c < KC:
        c1 = c
        while c1 < KC and tcuts[c1] == tcuts[c]:
            c1 += 1
        step = max(1, 1024 // tcuts[c])
        for s in range(c, c1, step):
            blocks.append((s, min(c1, s + step)))
        c = c1

    nc = bacc.Bacc("TRN2", target_bir_lowering=False, debug=False)

    rr_d = nc.dram_tensor("rr", [NPOLE], F32, kind="ExternalInput")
    th_d = nc.dram_tensor("theta", [NPOLE], F32, kind="ExternalInput")
    # pair-grouped blocks: [:, :, 0] = [x1|x3], [:, :, 1] = [x2|x4]
    xs_d = nc.dram_tensor("xs", [KC, 128, 2, 2, B, DSH], F16,
                          kind="ExternalInput")
    x0_d = nc.dram_tensor("x0", [B, DSH], F16, kind="ExternalInput")
    out_d = nc.dram_tensor("out", [DH, 2, njb, 128, B, 128], F16,
                           kind="ExternalOutput")

    with tile.TileContext(nc) as tc, ExitStack() as ctx:
        const = ctx.enter_context(tc.tile_pool(name="const", bufs=1))
        cfp = ctx.enter_context(tc.tile_pool(name="cfp", bufs=1))
        dictp = ctx.enter_context(tc.tile_pool(name="dictp", bufs=1))
        bwork = ctx.enter_context(tc.tile_pool(name="bwork", bufs=2))
        xp = ctx.enter_context(tc.tile_pool(name="xp", bufs=5))
        uvp = ctx.enter_context(tc.tile_pool(name="uvp", bufs=3))
        keepp = ctx.enter_context(tc.tile_pool(name="keepp", bufs=1))
        outp = ctx.enter_context(tc.tile_pool(name="outp", bufs=4))
        psp = ctx.enter_context(
            tc.tile_pool(name="ps", bufs=2, space=bass.MemorySpace.PSUM))

        # ---- tiny consts -------------------------------------------------
        rr_t = const.tile([128, KC], F32, tag="rr")
        th_t = const.tile([128, KC], F32, tag="th")
        nc.sync.dma_start(rr_t[:], rr_d[:].rearrange("(kc p) -> p kc", p=128))
        nc.sync.dma_start(th_t[:], th_d[:].rearrange("(kc p) -> p kc", p=128))
        x0_t = const.tile([1, B, DSH], F16, tag="x0")
        nc.sync.dma_start(x0_t[:], x0_d[:])

        hpi = const.tile([128, 1], F32, tag="hpi")
        nc.vector.memset(hpi[:], PI / 2)
        bigp = const.tile([128, 1], F32, tag="bigp")
        nc.vector.memset(bigp[:], RND_BIG)
        bigm = const.tile([128, 1], F32, tag="bigm")
        nc.vector.memset(bigm[:], -RND_BIG)
        ones_h = const.tile([1, 128], F16, tag="ones")
        nc.vector.memset(ones_h[:], 0.015625)  # half of 1/sqrt(T)
        th2p = const.tile([128, KC], F32, tag="th2p")
        nc.vector.tensor_scalar_mul(th2p[:], th_t[:], 1.0 / TWO_PI)

        # ---- closed-form column norms (same math as v1) ------------------
        sinth = const.tile([128, KC], F32, tag="sinth")
        nc.scalar.activation(sinth[:], th_t[:], AF.Sin, bias=0.0, scale=1.0)
        costh = const.tile([128, KC], F32, tag="costh")
        nc.scalar.activation(costh[:], th_t[:], AF.Sin, bias=hpi[:], scale=1.0)
        rs = const.tile([128, KC], F32, tag="rs")
        nc.vector.tensor_mul(rs[:], sinth[:], rr_t[:])
        maskB = const.tile([128, KC], F32, tag="maskB")
        nc.vector.tensor_scalar(maskB[:], rs[:], 0.0, None, op0=OP.is_gt)
        th2p = const.tile([128, KC], F32, tag="th2p")
        nc.vector.tensor_scalar_mul(th2p[:], th_t[:], 1.0 / TWO_PI)

        r2j = const.tile([128, 10, KC], F32, tag="r2j")
        nc.vector.tensor_mul(r2j[:, 0], rr_t[:], rr_t[:])
        for j in range(1, 10):
            nc.vector.tensor_mul(r2j[:, j], r2j[:, j - 1], r2j[:, j - 1])

        def cf(name):
            return cfp.tile([128, KC], F32, tag=name, name=name)

        R_ = r2j[:, 0]
        rt = cf("rt")
        nc.vector.tensor_mul(rt[:], r2j[:, 9], r2j[:, 9])
        omr = cf("omr")
        nc.vector.tensor_scalar(omr[:], rr_t[:], -1.0, 1.0, op0=OP.mult,
                                op1=OP.add)
        opr = cf("opr")
        nc.vector.tensor_scalar(opr[:], rr_t[:], 1.0, None, op0=OP.add)
        omR = cf("omR")
        nc.vector.tensor_mul(omR[:], omr[:], opr[:])
        ssq2 = cf("ssq2")
        nc.vector.scalar_tensor_tensor(ssq2[:], sinth[:], 2.0, sinth[:],
                                       op0=OP.mult, op1=OP.mult)
        s2t = cf("s2t")
        nc.vector.scalar_tensor_tensor(s2t[:], sinth[:], 2.0, costh[:],
                                       op0=OP.mult, op1=OP.mult)
        zim = cf("zim")
        nc.vector.tensor_mul(zim[:], R_, s2t[:])
        rmz = cf("rmz")
        nc.vector.tensor_mul(rmz[:], R_, ssq2[:])
        a1r = cf("a1r")
        nc.vector.tensor_add(a1r[:], omR[:], rmz[:])
        qq = cf("qq")
        nc.vector.tensor_scalar(qq[:], th2p[:], 2048.0, None, op0=OP.mult)
        qqr = cf("qqr")
        nc.vector.tensor_scalar(qqr[:], qq[:], RND_BIG, -RND_BIG, op0=OP.add,
                                op1=OP.add)
        dd = cf("dd")
        nc.vector.scalar_tensor_tensor(dd[:], qqr[:], -1.0, qq[:],
                                       op0=OP.mult, op1=OP.add)
        adt = cf("adt")
        nc.vector.tensor_scalar(adt[:].bitcast(I32), dd[:].bitcast(I32),
                                0x7FFFFFFF, None, op0=OP.bitwise_and)
        sT = cf("sT")
        nc.scalar.activation(sT[:], dd[:], AF.Sin, bias=0.0, scale=TWO_PI)
        cT = cf("cT")
        nc.scalar.activation(cT[:], adt[:], AF.Sin, bias=hpi[:], scale=-TWO_PI)
        zTr = cf("zTr")
        nc.vector.tensor_mul(zTr[:], rt[:], cT[:])
        zTi = cf("zTi")
        nc.vector.tensor_mul(zTi[:], rt[:], sT[:])
        omrt = cf("omrt")
        nc.vector.tensor_scalar(omrt[:], rt[:], -1.0, 1.0, op0=OP.mult,
                                op1=OP.add)
        rrec = cf("rrec")
        nc.vector.reciprocal(rrec[:], omR[:])
        s0_ = cf("s0_")
        nc.vector.tensor_mul(s0_[:], omrt[:], rrec[:])
        xx = cf("xx")
        nc.vector.tensor_scalar(xx[:], zTr[:], -1.0, 1.0, op0=OP.mult,
                                op1=OP.add)
        n1 = cf("n1")
        nc.vector.tensor_mul(n1[:], xx[:], a1r[:])
        n2 = cf("n2")
        nc.vector.tensor_mul(n2[:], zTi[:], zim[:])
        num = cf("num")
        nc.vector.tensor_add(num[:], n1[:], n2[:])
        dn1 = cf("dn1")
        nc.vector.tensor_mul(dn1[:], a1r[:], a1r[:])
        dn2 = cf("dn2")
        nc.vector.tensor_mul(dn2[:], zim[:], zim[:])
        den = cf("den")
        nc.vector.tensor_add(den[:], dn1[:], dn2[:])
        rden = cf("rden")
        nc.vector.reciprocal(rden[:], den[:])
        reC = cf("reC")
        nc.vector.tensor_mul(reC[:], num[:], rden[:])
        g2t = const.tile([128, 2, KC], F32, tag="g2t")
        nc.vector.tensor_add(g2t[:, 0], s0_[:], reC[:])
        nc.vector.tensor_scalar_mul(g2t[:, 0], g2t[:, 0], 0.5)
        nr1 = cf("nr1")
        nc.vector.tensor_mul(nr1[:], rt[:], a1r[:])
        nr2 = cf("nr2")
        nc.vector.tensor_mul(nr2[:], zTr[:], omR[:])
        nre = cf("nre")
        nc.vector.tensor_sub(nre[:], rmz[:], nr1[:])
        nc.vector.tensor_add(nre[:], nre[:], nr2[:])
        ni1 = cf("ni1")
        nc.vector.tensor_mul(ni1[:], zim[:], omrt[:])
        ni2 = cf("ni2")
        nc.vector.tensor_mul(ni2[:], zTi[:], omR[:])
        nim = cf("nim")
        nc.vector.tensor_sub(nim[:], ni2[:], ni1[:])
        dre = cf("dre")
        nc.vector.tensor_mul(dre[:], omR[:], a1r[:])
        dimp = cf("dimp")
        nc.vector.tensor_mul(dimp[:], omR[:], zim[:])
        m1_ = cf("m1_")
        nc.vector.tensor_mul(m1_[:], nre[:], dre[:])
        m2_ = cf("m2_")
        nc.vector.tensor_mul(m2_[:], nim[:], dimp[:])
        mnum = cf("mnum")
        nc.vector.tensor_sub(mnum[:], m1_[:], m2_[:])
        e1_ = cf("e1_")
        nc.vector.tensor_mul(e1_[:], dre[:], dre[:])
        e2_ = cf("e2_")
        nc.vector.tensor_mul(e2_[:], dimp[:], dimp[:])
        eden = cf("eden")
        nc.vector.tensor_add(eden[:], e1_[:], e2_[:])
        rede = cf("rede")
        nc.vector.reciprocal(rede[:], eden[:])
        nc.vector.tensor_mul(g2t[:, 1], mnum[:], rede[:])
        nc.vector.tensor_scalar_mul(g2t[:, 1], g2t[:, 1], 0.5)
        gcl = const.tile([128, 2, KC], F32, tag="gcl")
        nc.vector.tensor_scalar_max(gcl[:], g2t[:], 1e-30)
        y0i = const.tile([128, 2, KC], I32, tag="y0i")
        nc.vector.tensor_scalar(y0i[:], gcl[:].bitcast(I32), 1, None,
                                op0=OP.arith_shift_right)
        invgt = const.tile([128, 2, KC], F32, tag="invgt")
        y_t = invgt
        nc.vector.tensor_scalar(y_t[:].bitcast(I32), y0i[:], -1, RSQRT_MAGIC,
                                op0=OP.mult, op1=OP.add)
        yy = const.tile([128, 2, KC], F32, tag="yy")
        ff = const.tile([128, 2, KC], F32, tag="ff")
        for it in range(newton + 1):
            nc.vector.tensor_mul(yy[:], y_t[:], y_t[:])
            nc.vector.tensor_mul(yy[:], yy[:], gcl[:])
            nc.vector.tensor_scalar(ff[:], yy[:], -0.5, 1.5, op0=OP.mult,
                                    op1=OP.add)
            nc.vector.tensor_mul(y_t[:], y_t[:], ff[:])
        invgA = invgt[:, 0]
        # sclB = 2*invgB*mask (the 2 from sin(2pi d) = 2 sin(pi d) cos(pi d))
        sclB = const.tile([128, KC], F32, tag="sclB")
        nc.vector.scalar_tensor_tensor(sclB[:], invgt[:, 1], 2.0, maskB[:],
                                       op0=OP.mult, op1=OP.mult)

        # ---- t index (iota), shared by all build blocks ------------------
        tmax = max(tcuts)
        tf_i = const.tile([128, tmax], I32, tag="tfi")
        nc.gpsimd.iota(tf_i[:], pattern=[[1, tmax]], base=0,
                       channel_multiplier=0)
        tf32 = const.tile([128, tmax], F32, tag="tf32")
        nc.gpsimd.tensor_copy(tf32[:], tf_i[:])

        # ---- dictionary build (batched per uniform-tcut block) -----------
        adict = dictp.tile([128, SUMT], F16, tag="adict")
        bdict = dictp.tile([128, SUMT], F16, tag="bdict")
        pwA = dictp.tile([128, SUMT], F16, tag="pwA")

        def build_block(c0, c1):
            ng = c1 - c0
            tc_ = tcuts[c0]
            o0 = int(offs[c0])
            ncol = ng * tc_
            # pwA = r^t via geometric scan (fp32 state, fp16 out); the 1/G
            # normalization is applied later as a per-chunk ACT scale so the
            # scans do not wait on the closed-form-norm dependency chain
            seg = pwA[:, o0:o0 + ncol].rearrange("p (c t) -> p c t", t=tc_)
            nc.vector.memset(seg[:, :, 0], 1.0)
            for c in range(c0, c1):
                d0 = rr_t[:, c:c + 1].to_broadcast([128, tc_ - 1])
                nc.vector.tensor_tensor_scan(
                    pwA[:, offs[c] + 1:offs[c] + tc_], d0, d0,
                    1.0, op0=OP.mult, op1=OP.bypass)
            # half-angle: d = psi - round(psi); snh=sin(pi d), cnh=cos(pi d)
            psi = bwork.tile([128, ng, tc_], F32, tag="w1", name="psi")
            nc.gpsimd.tensor_tensor(
                psi[:], tf32[:, :tc_].unsqueeze(1).to_broadcast([128, ng, tc_]),
                th2p[:, c0:c1].unsqueeze(2).to_broadcast([128, ng, tc_]),
                op=OP.mult)
            qh = bwork.tile([128, ng, tc_], F32, tag="w2", name="qh")
            nc.scalar.activation(qh[:], psi[:], AF.Identity, bias=bigp[:],
                                 scale=1.0)
            nc.scalar.activation(qh[:], qh[:], AF.Identity, bias=bigm[:],
                                 scale=1.0)
            dt_ = bwork.tile([128, ng, tc_], F32, tag="w3", name="dt")
            nc.gpsimd.tensor_sub(dt_[:], psi[:], qh[:])
            snh = bwork.tile([128, ng, tc_], F16, tag="w4", name="snh")
            nc.scalar.activation(snh[:], dt_[:], AF.Sin, bias=0.0, scale=PI)
            cnh = bwork.tile([128, ng, tc_], F16, tag="w5", name="cnh")
            nc.scalar.activation(cnh[:], dt_[:], AF.Sin, bias=hpi[:],
                                 scale=-PI)
            # cos(2pi d) = 1 - 2 snh^2 ; sin(2pi d) = 2 snh cnh (2 in sclB)
            sq = bwork.tile([128, ng, tc_], F16, tag="w6", name="sq")
            nc.vector.tensor_tensor(sq[:], snh[:], snh[:], op=OP.mult)
            c2 = bwork.tile([128, ng, tc_], F16, tag="w7", name="c2")
            nc.scalar.activation(c2[:], sq[:], AF.Identity, bias=1.0,
                                 scale=-2.0)
            snc = bwork.tile([128, ng, tc_], F16, tag="w8", name="snc")
            nc.vector.tensor_tensor(snc[:], snh[:], cnh[:], op=OP.mult)
            aseg = adict[:, o0:o0 + ncol].rearrange("p (c t) -> p c t", t=tc_)
            bseg = bdict[:, o0:o0 + ncol].rearrange("p (c t) -> p c t", t=tc_)
            pa = pwA[:, o0:o0 + ncol].rearrange("p (c t) -> p c t", t=tc_)
            nc.vector.tensor_tensor(aseg[:], c2[:], pa[:], op=OP.mult)
            nc.vector.tensor_tensor(bseg[:], snc[:], pa[:], op=OP.mult)
            # late normalization: per-chunk 1/G scales (ACT, per-partition AP)
            for c in range(c0, c1):
                asl = adict[:, offs[c]:offs[c] + tc_]
                nc.scalar.activation(asl, asl, AF.Identity, bias=0.0,
                                     scale=invgA[:, c:c + 1])
                bsl = bdict[:, offs[c]:offs[c] + tc_]
                nc.scalar.activation(bsl, bsl, AF.Identity, bias=0.0,
                                     scale=sclB[:, c:c + 1])

        # ---- x load + combines ------------------------------------------
        def load_combine(c, keep=False):
            pool = keepp if keep else uvp
            xt = xp.tile([128, 2, 2, B, DSH], F16, tag="xt", name=f"xt{c}")
            nc.sync.dma_start(xt[:], xs_d[c])
            sfx = f"k{c}" if keep else "s"
            uw = pool.tile([128, 2, B, DSH], F16, tag=f"uw{sfx}", name=f"uw{c}")
            vz = pool.tile([128, 2, B, DSH], F16, tag=f"vz{sfx}", name=f"vz{c}")
            nc.vector.tensor_add(uw[:], xt[:, 0], xt[:, 1])
            nc.vector.tensor_sub(vz[:], xt[:, 0], xt[:, 1])
            return uw[:, 0], vz[:, 0], uw[:, 1], vz[:, 1]

        long_chunks = [c for c in range(KC) if jcuts[c] > 128]
        kept = {}

        built = set()

        def ensure_block(c):
            for (c0, c1) in blocks:
                if c0 <= c < c1 and (c0, c1) not in built:
                    built.add((c0, c1))
                    build_block(c0, c1)

        ensure_block(0)
        if KC > 1:
            ensure_block(1)
        for c in long_chunks[:L_RETAIN]:
            kept[c] = load_combine(c, keep=True)

        def stat_ap(dic, c, jb, par, h):
            seg = dic[:, offs[c]:offs[c] + tcuts[c]].rearrange(
                "p (j two) -> p j two", two=2)
            return seg[:, 128 * jb:128 * jb + h, par]

        def chunk_mms(c, jb, uvwz, psums, first, last):
            u, v, w, z = uvwz
            h = min(128, jcuts[c] - 128 * jb)
            for dh in range(DH):
                dsl = slice(dh * 128, (dh + 1) * 128)
                for par, (mA, mB) in enumerate(((u, w), (v, z))):
                    ps = psums[(dh, par)]
                    if first:
                        nc.tensor.matmul(ps[:], ones_h[:], x0_t[:, :, dsl],
                                         start=True, stop=False)
                    nc.tensor.matmul(ps[0:h], stat_ap(adict, c, jb, par, h),
                                     mA[:, :, dsl], start=False, stop=False)
                    nc.tensor.matmul(ps[0:h], stat_ap(bdict, c, jb, par, h),
                                     mB[:, :, dsl], start=False, stop=False)
                    if last:
                        nc.tensor.matmul(ps[:], ones_h[:], x0_t[:, :, dsl],
                                         start=False, stop=True)

        def open_psums():
            return {(dh, par): psp.tile([128, B, 128], F32,
                                        tag=f"ps{dh}{par}",
                                        name=f"ps{dh}{par}")
                    for dh in range(DH) for par in range(2)}

        def run_pass(jb, chunks, interleave_build=False):
            psums = open_psums()
            if not chunks:
                # ones-term only (general-input safety; never hit when
                # some chunk reaches this j-block)
                for dh in range(DH):
                    dsl = slice(dh * 128, (dh + 1) * 128)
                    for par in range(2):
                        ps = psums[(dh, par)]
                        nc.tensor.matmul(ps[:], ones_h[:], x0_t[:, :, dsl],
                                         start=True, stop=False)
                        nc.tensor.matmul(ps[:], ones_h[:], x0_t[:, :, dsl],
                                         start=False, stop=True)
            else:
                for i, c in enumerate(chunks):
                    if interleave_build:
                        ensure_block(c)
                    uvwz = kept.get(c)
                    if uvwz is None:
                        uvwz = load_combine(c)
                    chunk_mms(c, jb, uvwz, psums, i == 0, i == len(chunks) - 1)
            # drain: ACT copy psum -> sbuf fp16, DMA out on scalar queue
            for (dh, par), ps in psums.items():
                ob = outp.tile([128, B, 128], F16, tag="ob",
                               name=f"ob{jb}{dh}{par}")
                nc.scalar.activation(ob[:], ps[:], AF.Identity, bias=0.0,
                                     scale=1.0)
                nc.scalar.dma_start(out_d[dh, par, jb], ob[:])

        # main jb0 pass streams all chunks (dict blocks built just-in-time
        # so DVE work interleaves with the stream); the extra j-blocks only
        # touch the retained long chunks and run afterwards
        run_pass(0, list(range(KC)), interleave_build=True)
        for jb in range(1, njb):
            run_pass(jb, [c for c in range(KC) if jcuts[c] > 128 * jb])

    nc.compile()
    return nc


_NC_CACHE = {}


def _get_nc(jcuts):
    key = tuple(jcuts)
    if key not in _NC_CACHE:
        _NC_CACHE[key] = build_kernel_nc(key)
    return _NC_CACHE[key]


def _jcuts(rr_sorted):
    L = float(np.log(1.0 / DELTA))
    jc = []
    for c in range(KC):
        rmax = float(rr_sorted[c * 128])
        neg = max(-np.log(max(rmax, 1e-30)), 1e-12)
        j = L / (2.0 * neg)
        j = int(np.ceil(j / JGRAN)) * JGRAN
        jc.append(int(np.clip(j, JGRAN, 512)))
    return jc


def kernel(rr, theta, x, trace=False, trace_kwargs=None):
    rr = np.ascontiguousarray(np.asarray(rr, dtype=np.float32))
    theta = np.ascontiguousarray(np.asarray(theta, dtype=np.float32))
    x = np.asarray(x, dtype=np.float32)
    Bx, KTOT, D = x.shape
    dsh = D // N_CORES

    perm = np.argsort(-rr, kind="stable")
    rr_s = np.ascontiguousarray(rr[perm])
    th_s = np.ascontiguousarray(theta[perm])
    jcuts = _jcuts(rr_s)
    nc = _get_nc(jcuts)

    x16 = x.astype(np.float16)
    # pair-grouped block order (x1,x3),(x2,x4) so one tensor_tensor makes
    # [U|W] and one makes [V|Z]
    xb = x16[:, 1:, :].reshape(Bx, 4, NPOLE, D)[:, [0, 2, 1, 3], :, :]
    xb = xb[:, :, perm, :]
    in_maps = []
    for core in range(N_CORES):
        dsl = slice(core * dsh, (core + 1) * dsh)
        t = xb[:, :, :, dsl]                      # [b, blk(x1,x3,x2,x4), pole, d]
        xs = np.ascontiguousarray(
            t.transpose(2, 1, 0, 3).reshape(KC, 128, 2, 2, Bx, dsh))
        in_maps.append({
            "rr": rr_s, "theta": th_s, "xs": xs,
            "x0": np.ascontiguousarray(x16[:, 0, dsl]),
        })
    kw = {}
    if trace:
        kw = {"trace": True, "trace_kwargs": trace_kwargs or {}}
    res = bass_utils.run_bass_kernel_spmd(nc, in_maps,
                                          core_ids=list(range(N_CORES)), **kw)
    out = np.empty((Bx, T, D), dtype=np.float32)
    for core in range(N_CORES):
        buf = res.results[core]["out"]            # [dh, par, jb, j, b, d]
        for dh in range(dsh // 128):
            dcols = slice(core * dsh + dh * 128, core * dsh + (dh + 1) * 128)
            for par in range(2):
                for jb in range(4):
                    tsl = slice(256 * jb + par, 256 * jb + 256, 2)
                    out[:, tsl, dcols] = np.moveaxis(
                        buf[dh, par, jb], 1, 0).astype(np.float32)
    if trace:
        return out, res
    return out


# revision 4
# speedup vs baseline: 1.1762x; 1.1762x over previous
"""Trainium2 Bass kernel for nn_Decoder_481036337511 — v2.

out[b,t,d] = sum_k dic[t,k] x[b,k,d];  dic = normalized real dictionary
[T=1024, 1+4*4096] built on-device from rr/theta; x is [4, 16385, 2048].

Strategy (8 cores data-parallel on D, 256 cols each):
  - Parity trick: dic blocks [ones, A, S*A, B, S*B] with S=diag((-1)^t);
    out[even] = A@U + B@W + ones-term, out[odd] = A@V + B@Z with
    U=x1+x2, V=x1-x2, W=x3+x4, Z=x3-x4 (combines on DVE/Pool, fp16).
  - Truncation: poles sorted by r DESC on host (contraction order is
    free). After unit-normalization a pole's column decays like r^t, so
    chunk c of 128 sorted poles only needs t < tcut_c with
    tcut = ln(1/delta)/ln(1/rmax); only the first chunk needs full T.
    Exact tail error computed host-side is ~1e-5 for delta=1e-3.
  - Orientation: dict chunks are the matmul STATIONARY ([128 poles,
    <=128 j] with parity-interleaved stride-2 cols), x-combines are the
    MOVING ([128, 4b x 128d] = 512 cols per LDWEIGHTS). PSUM out is
    [j, b, d]; the ones-column term is folded in as a 1-partition
    stationary matmul (0.015625 * 2 = 1/sqrt(1024)).
  - Dict built on-device in t-interleaved layout: r^t via
    tensor_tensor_scan (geometric recurrence, initial = 1/G from the
    closed-form norms), angles via iota*theta/2pi with the fp32
    big-constant round trick, sin/cos via ACT Sin table. Build work is
    batched over uniform-tcut chunk groups with stride-0 broadcast APs.
  - x shipped fp16 in matmul-ready layout [chunk, pole, blk, b, d]
    (halves HBM traffic; host does layout/cast only, no arithmetic).
    Output written as fp16 [dh, par, jb, j, b, d] tiles, host casts to
    fp32 and interleaves parities.
"""

import numpy as np
from contextlib import ExitStack

import concourse.bass as bass
import concourse.bacc as bacc
import concourse.mybir as mybir
from concourse import tile
from concourse import bass_utils

F32 = mybir.dt.float32
F16 = mybir.dt.float16
I32 = mybir.dt.int32
AF = mybir.ActivationFunctionType
OP = mybir.AluOpType

N_CORES = 8
PI = float(np.pi)
TWO_PI = float(2 * np.pi)
RSQRT_MAGIC = 0x5F3759DF
RND_BIG = 12582912.0  # 2^23 + 2^22: (q + BIG) - BIG == round(q), |q| < 2^22
NPOLE = 4096
KC = 32
B = 4
DSH = 256
T = 1024
DELTA = 1e-3  # truncation tail target (achieved tail err ~1e-5)
JGRAN = 64
L_RETAIN = 4  # max long chunks whose combines stay resident


def build_kernel_nc(jcuts, newton=1):
    """Emit the per-core Bass program for the given per-chunk j-cuts."""
    jcuts = list(jcuts)
    tcuts = [2 * j for j in jcuts]
    offs = np.concatenate([[0], np.cumsum(tcuts)]).astype(int)
    SUMT = int(offs[-1])
    maxjb = max(int(np.ceil(j / 128)) for j in jcuts)
    njb = 4  # T/256: output always covers 4 j-blocks
    DH = DSH // 128

    # build blocks: runs of equal tcut, split so Ng*tcut <= 1024
    blocks = []
    c = 0
    while c < KC:
        c1 = c
        while c1 < KC and tcuts[c1] == tcuts[c]:
            c1 += 1
        step = max(1, 1024 // tcuts[c])
        for s in range(c, c1, step):
            blocks.append((s, min(c1, s + step)))
        c = c1

    nc = bacc.Bacc("TRN2", target_bir_lowering=False, debug=False)

    rr_d = nc.dram_tensor("rr", [NPOLE], F32, kind="ExternalInput")
    th_d = nc.dram_tensor("theta", [NPOLE], F32, kind="ExternalInput")
    # pair-grouped blocks: [:, :, 0] = [x1|x3], [:, :, 1] = [x2|x4]
    xs_d = nc.dram_tensor("xs", [KC, 128, 2, 2, B, DSH], F16,
                          kind="ExternalInput")
    x0_d = nc.dram_tensor("x0", [B, DSH], F16, kind="ExternalInput")
    out_d = nc.dram_tensor("out", [DH, 2, njb, 128, B, 128], F16,
                           kind="ExternalOutput")

    with tile.TileContext(nc) as tc, ExitStack() as ctx:
        const = ctx.enter_context(tc.tile_pool(name="const", bufs=1))
        cfp = ctx.enter_context(tc.tile_pool(name="cfp", bufs=1))
        dictp = ctx.enter_context(tc.tile_pool(name="dictp", bufs=1))
        bwork = ctx.enter_context(tc.tile_pool(name="bwork", bufs=2))
        xp = ctx.enter_context(tc.tile_pool(name="xp", bufs=6))
        uvp = ctx.enter_context(tc.tile_pool(name="uvp", bufs=4))
        keepp = ctx.enter_context(tc.tile_pool(name="keepp", bufs=1))
        outp = ctx.enter_context(tc.tile_pool(name="outp", bufs=4))
        psp = ctx.enter_context(
            tc.tile_pool(name="ps", bufs=1, space=bass.MemorySpace.PSUM))
        psxp = ctx.enter_context(
            tc.tile_pool(name="psx", bufs=1, space=bass.MemorySpace.PSUM))

        # ---- tiny consts -------------------------------------------------
        rr_t = const.tile([128, KC], F32, tag="rr")
        th_t = const.tile([128, KC], F32, tag="th")
        nc.sync.dma_start(rr_t[:], rr_d[:].rearrange("(kc p) -> p kc", p=128))
        nc.sync.dma_start(th_t[:], th_d[:].rearrange("(kc p) -> p kc", p=128))
        x0_t = const.tile([1, B, DSH], F16, tag="x0")
        nc.sync.dma_start(x0_t[:], x0_d[:])

        hpi = const.tile([128, 1], F32, tag="hpi")
        nc.vector.memset(hpi[:], PI / 2)
        bigp = const.tile([128, 1], F32, tag="bigp")
        nc.vector.memset(bigp[:], RND_BIG)
        bigm = const.tile([128, 1], F32, tag="bigm")
        nc.vector.memset(bigm[:], -RND_BIG)
        ones_h = const.tile([1, 128], F16, tag="ones")
        nc.vector.memset(ones_h[:], 0.015625)  # half of 1/sqrt(T)
        th2p = const.tile([128, KC], F32, tag="th2p")
        nc.vector.tensor_scalar_mul(th2p[:], th_t[:], 1.0 / TWO_PI)

        # ---- closed-form column norms (same math as v1) ------------------
        sinth = const.tile([128, KC], F32, tag="sinth")
        nc.scalar.activation(sinth[:], th_t[:], AF.Sin, bias=0.0, scale=1.0)
        costh = const.tile([128, KC], F32, tag="costh")
        nc.scalar.activation(costh[:], th_t[:], AF.Sin, bias=hpi[:], scale=1.0)
        rs = const.tile([128, KC], F32, tag="rs")
        nc.vector.tensor_mul(rs[:], sinth[:], rr_t[:])
        maskB = const.tile([128, KC], F32, tag="maskB")
        nc.vector.tensor_scalar(maskB[:], rs[:], 0.0, None, op0=OP.is_gt)
        th2p = const.tile([128, KC], F32, tag="th2p")
        nc.vector.tensor_scalar_mul(th2p[:], th_t[:], 1.0 / TWO_PI)

        r2j = const.tile([128, 10, KC], F32, tag="r2j")
        nc.vector.tensor_mul(r2j[:, 0], rr_t[:], rr_t[:])
        for j in range(1, 10):
            nc.vector.tensor_mul(r2j[:, j], r2j[:, j - 1], r2j[:, j - 1])

        def cf(name):
            return cfp.tile([128, KC], F32, tag=name, name=name)

        R_ = r2j[:, 0]
        rt = cf("rt")
        nc.vector.tensor_mul(rt[:], r2j[:, 9], r2j[:, 9])
        omr = cf("omr")
        nc.vector.tensor_scalar(omr[:], rr_t[:], -1.0, 1.0, op0=OP.mult,
                                op1=OP.add)
        opr = cf("opr")
        nc.vector.tensor_scalar(opr[:], rr_t[:], 1.0, None, op0=OP.add)
        omR = cf("omR")
        nc.vector.tensor_mul(omR[:], omr[:], opr[:])
        ssq2 = cf("ssq2")
        nc.vector.scalar_tensor_tensor(ssq2[:], sinth[:], 2.0, sinth[:],
                                       op0=OP.mult, op1=OP.mult)
        s2t = cf("s2t")
        nc.vector.scalar_tensor_tensor(s2t[:], sinth[:], 2.0, costh[:],
                                       op0=OP.mult, op1=OP.mult)
        zim = cf("zim")
        nc.vector.tensor_mul(zim[:], R_, s2t[:])
        rmz = cf("rmz")
        nc.vector.tensor_mul(rmz[:], R_, ssq2[:])
        a1r = cf("a1r")
        nc.vector.tensor_add(a1r[:], omR[:], rmz[:])
        qq = cf("qq")
        nc.vector.tensor_scalar(qq[:], th2p[:], 2048.0, None, op0=OP.mult)
        qqr = cf("qqr")
        nc.vector.tensor_scalar(qqr[:], qq[:], RND_BIG, -RND_BIG, op0=OP.add,
                                op1=OP.add)
        dd = cf("dd")
        nc.vector.scalar_tensor_tensor(dd[:], qqr[:], -1.0, qq[:],
                                       op0=OP.mult, op1=OP.add)
        adt = cf("adt")
        nc.vector.tensor_scalar(adt[:].bitcast(I32), dd[:].bitcast(I32),
                                0x7FFFFFFF, None, op0=OP.bitwise_and)
        sT = cf("sT")
        nc.scalar.activation(sT[:], dd[:], AF.Sin, bias=0.0, scale=TWO_PI)
        cT = cf("cT")
        nc.scalar.activation(cT[:], adt[:], AF.Sin, bias=hpi[:], scale=-TWO_PI)
        zTr = cf("zTr")
        nc.vector.tensor_mul(zTr[:], rt[:], cT[:])
        zTi = cf("zTi")
        nc.vector.tensor_mul(zTi[:], rt[:], sT[:])
        omrt = cf("omrt")
        nc.vector.tensor_scalar(omrt[:], rt[:], -1.0, 1.0, op0=OP.mult,
                                op1=OP.add)
        rrec = cf("rrec")
        nc.vector.reciprocal(rrec[:], omR[:])
        s0_ = cf("s0_")
        nc.vector.tensor_mul(s0_[:], omrt[:], rrec[:])
        xx = cf("xx")
        nc.vector.tensor_scalar(xx[:], zTr[:], -1.0, 1.0, op0=OP.mult,
                                op1=OP.add)
        n1 = cf("n1")
        nc.vector.tensor_mul(n1[:], xx[:], a1r[:])
        n2 = cf("n2")
        nc.vector.tensor_mul(n2[:], zTi[:], zim[:])
        num = cf("num")
        nc.vector.tensor_add(num[:], n1[:], n2[:])
        dn1 = cf("dn1")
        nc.vector.tensor_mul(dn1[:], a1r[:], a1r[:])
        dn2 = cf("dn2")
        nc.vector.tensor_mul(dn2[:], zim[:], zim[:])
        den = cf("den")
        nc.vector.tensor_add(den[:], dn1[:], dn2[:])
        rden = cf("rden")
        nc.vector.reciprocal(rden[:], den[:])
        reC = cf("reC")
        nc.vector.tensor_mul(reC[:], num[:], rden[:])
        g2t = const.tile([128, 2, KC], F32, tag="g2t")
        nc.vector.tensor_add(g2t[:, 0], s0_[:], reC[:])
        nc.vector.tensor_scalar_mul(g2t[:, 0], g2t[:, 0], 0.5)
        nr1 = cf("nr1")
        nc.vector.tensor_mul(nr1[:], rt[:], a1r[:])
        nr2 = cf("nr2")
        nc.vector.tensor_mul(nr2[:], zTr[:], omR[:])
        nre = cf("nre")
        nc.vector.tensor_sub(nre[:], rmz[:], nr1[:])
        nc.vector.tensor_add(nre[:], nre[:], nr2[:])
        ni1 = cf("ni1")
        nc.vector.tensor_mul(ni1[:], zim[:], omrt[:])
        ni2 = cf("ni2")
        nc.vector.tensor_mul(ni2[:], zTi[:], omR[:])
        nim = cf("nim")
        nc.vector.tensor_sub(nim[:], ni2[:], ni1[:])
        dre = cf("dre")
        nc.vector.tensor_mul(dre[:], omR[:], a1r[:])
        dimp = cf("dimp")
        nc.vector.tensor_mul(dimp[:], omR[:], zim[:])
        m1_ = cf("m1_")
        nc.vector.tensor_mul(m1_[:], nre[:], dre[:])
        m2_ = cf("m2_")
        nc.vector.tensor_mul(m2_[:], nim[:], dimp[:])
        mnum = cf("mnum")
        nc.vector.tensor_sub(mnum[:], m1_[:], m2_[:])
        e1_ = cf("e1_")
        nc.vector.tensor_mul(e1_[:], dre[:], dre[:])
        e2_ = cf("e2_")
        nc.vector.tensor_mul(e2_[:], dimp[:], dimp[:])
        eden = cf("eden")
        nc.vector.tensor_add(eden[:], e1_[:], e2_[:])
        rede = cf("rede")
        nc.vector.reciprocal(rede[:], eden[:])
        nc.vector.tensor_mul(g2t[:, 1], mnum[:], rede[:])
        nc.vector.tensor_scalar_mul(g2t[:, 1], g2t[:, 1], 0.5)
        gcl = const.tile([128, 2, KC], F32, tag="gcl")
        nc.vector.tensor_scalar_max(gcl[:], g2t[:], 1e-30)
        y0i = const.tile([128, 2, KC], I32, tag="y0i")
        nc.vector.tensor_scalar(y0i[:], gcl[:].bitcast(I32), 1, None,
                                op0=OP.arith_shift_right)
        invgt = const.tile([128, 2, KC], F32, tag="invgt")
        y_t = invgt
        nc.vector.tensor_scalar(y_t[:].bitcast(I32), y0i[:], -1, RSQRT_MAGIC,
                                op0=OP.mult, op1=OP.add)
        yy = const.tile([128, 2, KC], F32, tag="yy")
        ff = const.tile([128, 2, KC], F32, tag="ff")
        for it in range(newton + 1):
            nc.vector.tensor_mul(yy[:], y_t[:], y_t[:])
            nc.vector.tensor_mul(yy[:], yy[:], gcl[:])
            nc.vector.tensor_scalar(ff[:], yy[:], -0.5, 1.5, op0=OP.mult,
                                    op1=OP.add)
            nc.vector.tensor_mul(y_t[:], y_t[:], ff[:])
        invgA = invgt[:, 0]
        # sclB = 2*invgB*mask (the 2 from sin(2pi d) = 2 sin(pi d) cos(pi d))
        sclB = const.tile([128, KC], F32, tag="sclB")
        nc.vector.scalar_tensor_tensor(sclB[:], invgt[:, 1], 2.0, maskB[:],
                                       op0=OP.mult, op1=OP.mult)

        # ---- t index (iota), shared by all build blocks ------------------
        tmax = max(tcuts)
        tf_i = const.tile([128, tmax], I32, tag="tfi")
        nc.gpsimd.iota(tf_i[:], pattern=[[1, tmax]], base=0,
                       channel_multiplier=0)
        tf32 = const.tile([128, tmax], F32, tag="tf32")
        nc.gpsimd.tensor_copy(tf32[:], tf_i[:])

        # ---- dictionary build (batched per uniform-tcut block) -----------
        adict = dictp.tile([128, SUMT], F16, tag="adict")
        bdict = dictp.tile([128, SUMT], F16, tag="bdict")
        pwA = dictp.tile([128, SUMT], F16, tag="pwA")

        def build_block(c0, c1):
            ng = c1 - c0
            tc_ = tcuts[c0]
            o0 = int(offs[c0])
            ncol = ng * tc_
            # pwA = r^t via geometric scan (fp32 state, fp16 out); the 1/G
            # normalization is applied later as a per-chunk ACT scale so the
            # scans do not wait on the closed-form-norm dependency chain
            seg = pwA[:, o0:o0 + ncol].rearrange("p (c t) -> p c t", t=tc_)
            nc.vector.memset(seg[:, :, 0], 1.0)
            for c in range(c0, c1):
                d0 = rr_t[:, c:c + 1].to_broadcast([128, tc_ - 1])
                nc.vector.tensor_tensor_scan(
                    pwA[:, offs[c] + 1:offs[c] + tc_], d0, d0,
                    1.0, op0=OP.mult, op1=OP.bypass)
            # half-angle: d = psi - round(psi); snh=sin(pi d), cnh=cos(pi d)
            psi = bwork.tile([128, ng, tc_], F32, tag="w1", name="psi")
            nc.gpsimd.tensor_tensor(
                psi[:], tf32[:, :tc_].unsqueeze(1).to_broadcast([128, ng, tc_]),
                th2p[:, c0:c1].unsqueeze(2).to_broadcast([128, ng, tc_]),
                op=OP.mult)
            qh = bwork.tile([128, ng, tc_], F32, tag="w2", name="qh")
            nc.scalar.activation(qh[:], psi[:], AF.Identity, bias=bigp[:],
                                 scale=1.0)
            nc.scalar.activation(qh[:], qh[:], AF.Identity, bias=bigm[:],
                                 scale=1.0)
            dt_ = bwork.tile([128, ng, tc_], F32, tag="w3", name="dt")
            nc.gpsimd.tensor_sub(dt_[:], psi[:], qh[:])
            snh = bwork.tile([128, ng, tc_], F16, tag="w4", name="snh")
            nc.scalar.activation(snh[:], dt_[:], AF.Sin, bias=0.0, scale=PI)
            cnh = bwork.tile([128, ng, tc_], F16, tag="w5", name="cnh")
            nc.scalar.activation(cnh[:], dt_[:], AF.Sin, bias=hpi[:],
                                 scale=-PI)
            # cos(2pi d) = 1 - 2 snh^2 ; sin(2pi d) = 2 snh cnh (2 in sclB)
            sq = bwork.tile([128, ng, tc_], F16, tag="w6", name="sq")
            nc.vector.tensor_tensor(sq[:], snh[:], snh[:], op=OP.mult)
            c2 = bwork.tile([128, ng, tc_], F16, tag="w7", name="c2")
            nc.scalar.activation(c2[:], sq[:], AF.Identity, bias=1.0,
                                 scale=-2.0)
            snc = bwork.tile([128, ng, tc_], F16, tag="w8", name="snc")
            nc.vector.tensor_tensor(snc[:], snh[:], cnh[:], op=OP.mult)
            aseg = adict[:, o0:o0 + ncol].rearrange("p (c t) -> p c t", t=tc_)
            bseg = bdict[:, o0:o0 + ncol].rearrange("p (c t) -> p c t", t=tc_)
            pa = pwA[:, o0:o0 + ncol].rearrange("p (c t) -> p c t", t=tc_)
            nc.vector.tensor_tensor(aseg[:], c2[:], pa[:], op=OP.mult)
            nc.vector.tensor_tensor(bseg[:], snc[:], pa[:], op=OP.mult)
            # late normalization: per-chunk 1/G scales (ACT, per-partition AP)
            for c in range(c0, c1):
                asl = adict[:, offs[c]:offs[c] + tc_]
                nc.scalar.activation(asl, asl, AF.Identity, bias=0.0,
                                     scale=invgA[:, c:c + 1])
                bsl = bdict[:, offs[c]:offs[c] + tc_]
                nc.scalar.activation(bsl, bsl, AF.Identity, bias=0.0,
                                     scale=sclB[:, c:c + 1])

        # ---- x load + combines ------------------------------------------
        def load_combine(c, keep=False):
            pool = keepp if keep else uvp
            xt = xp.tile([128, 2, 2, B, DSH], F16, tag="xt", name=f"xt{c}")
            nc.sync.dma_start(xt[:], xs_d[c])
            sfx = f"k{c}" if keep else "s"
            uw = pool.tile([128, 2, B, DSH], F16, tag=f"uw{sfx}", name=f"uw{c}")
            vz = pool.tile([128, 2, B, DSH], F16, tag=f"vz{sfx}", name=f"vz{c}")
            nc.vector.tensor_add(uw[:], xt[:, 0], xt[:, 1])
            nc.vector.tensor_sub(vz[:], xt[:, 0], xt[:, 1])
            return uw[:, 0], vz[:, 0], uw[:, 1], vz[:, 1]

        long_chunks = [c for c in range(KC) if jcuts[c] > 128]
        kept = {}

        built = set()

        def ensure_block(c):
            for (c0, c1) in blocks:
                if c0 <= c < c1 and (c0, c1) not in built:
                    built.add((c0, c1))
                    build_block(c0, c1)

        ensure_block(0)
        if KC > 1:
            ensure_block(1)
        for c in long_chunks[:L_RETAIN]:
            kept[c] = load_combine(c, keep=True)

        def stat_ap(dic, c, jb, par, h):
            seg = dic[:, offs[c]:offs[c] + tcuts[c]].rearrange(
                "p (j two) -> p j two", two=2)
            return seg[:, 128 * jb:128 * jb + h, par]

        def chunk_mms(c, jb, uvwz, psums, first, last):
            u, v, w, z = uvwz
            h = min(128, jcuts[c] - 128 * jb)
            for dh in range(DH):
                dsl = slice(dh * 128, (dh + 1) * 128)
                for par, (mA, mB) in enumerate(((u, w), (v, z))):
                    ps = psums[(dh, par)]
                    if first:
                        nc.tensor.matmul(ps[:], ones_h[:], x0_t[:, :, dsl],
                                         start=True, stop=False)
                    nc.tensor.matmul(ps[0:h], stat_ap(adict, c, jb, par, h),
                                     mA[:, :, dsl], start=False, stop=False)
                    nc.tensor.matmul(ps[0:h], stat_ap(bdict, c, jb, par, h),
                                     mB[:, :, dsl], start=False, stop=False)
                    if last:
                        nc.tensor.matmul(ps[:], ones_h[:], x0_t[:, :, dsl],
                                         start=False, stop=True)

        def open_psums(pool):
            return {(dh, par): pool.tile([128, B, 128], F32,
                                         tag=f"ps{dh}{par}",
                                         name=f"ps{dh}{par}")
                    for dh in range(DH) for par in range(2)}

        def run_pass(jb, chunks, pool=None, interleave_build=False,
                     after_first=None):
            psums = open_psums(pool or psp)
            if not chunks:
                # ones-term only (general-input safety; never hit when
                # some chunk reaches this j-block)
                for dh in range(DH):
                    dsl = slice(dh * 128, (dh + 1) * 128)
                    for par in range(2):
                        ps = psums[(dh, par)]
                        nc.tensor.matmul(ps[:], ones_h[:], x0_t[:, :, dsl],
                                         start=True, stop=False)
                        nc.tensor.matmul(ps[:], ones_h[:], x0_t[:, :, dsl],
                                         start=False, stop=True)
            else:
                for i, c in enumerate(chunks):
                    if interleave_build:
                        ensure_block(c)
                        ensure_block(min(c + 6, KC - 1), pre_only=True)
                    uvwz = kept.get(c)
                    if uvwz is None:
                        uvwz = load_combine(c)
                    chunk_mms(c, jb, uvwz, psums, i == 0, i == len(chunks) - 1)
                    if i == 0 and after_first is not None:
                        after_first()
            # drain: ACT copy psum -> sbuf fp16, DMA out on scalar queue
            for (dh, par), ps in psums.items():
                ob = outp.tile([128, B, 128], F16, tag="ob",
                               name=f"ob{jb}{dh}{par}")
                nc.scalar.activation(ob[:], ps[:], AF.Identity, bias=0.0,
                                     scale=1.0)
                nc.scalar.dma_start(out_d[dh, par, jb], ob[:])

        # main jb0 pass streams all chunks (dict blocks built just-in-time
        # so DVE work interleaves with the stream). The extra j-blocks only
        # touch the retained long chunks; they run INSIDE the stream right
        # after chunk 0 (own psum bank set), filling the PE's early wait
        # gap and removing the drain tail.
        def extras():
            for jb in range(1, njb):
                run_pass(jb, [c for c in range(KC) if jcuts[c] > 128 * jb],
                         pool=psxp)

        run_pass(0, list(range(KC)), interleave_build=True,
                 after_first=extras)

    nc.compile()
    return nc


_NC_CACHE = {}


def _get_nc(jcuts):
    key = tuple(jcuts)
    if key not in _NC_CACHE:
        _NC_CACHE[key] = build_kernel_nc(key)
    return _NC_CACHE[key]


def _jcuts(rr_sorted):
    L = float(np.log(1.0 / DELTA))
    jc = []
    for c in range(KC):
        rmax = float(rr_sorted[c * 128])
        neg = max(-np.log(max(rmax, 1e-30)), 1e-12)
        j = L / (2.0 * neg)
        j = int(np.ceil(j / JGRAN)) * JGRAN
        jc.append(int(np.clip(j, JGRAN, 512)))
    return jc


def kernel(rr, theta, x, trace=False, trace_kwargs=None):
    rr = np.ascontiguousarray(np.asarray(rr, dtype=np.float32))
    theta = np.ascontiguousarray(np.asarray(theta, dtype=np.float32))
    x = np.asarray(x, dtype=np.float32)
    Bx, KTOT, D = x.shape
    dsh = D // N_CORES

    perm = np.argsort(-rr, kind="stable")
    rr_s = np.ascontiguousarray(rr[perm])
    th_s = np.ascontiguousarray(theta[perm])
    jcuts = _jcuts(rr_s)
    nc = _get_nc(jcuts)

    x16 = x.astype(np.float16)
    # pair-grouped block order (x1,x3),(x2,x4) so one tensor_tensor makes
    # [U|W] and one makes [V|Z]
    xb = x16[:, 1:, :].reshape(Bx, 4, NPOLE, D)[:, [0, 2, 1, 3], :, :]
    xb = xb[:, :, perm, :]
    in_maps = []
    for core in range(N_CORES):
        dsl = slice(core * dsh, (core + 1) * dsh)
        t = xb[:, :, :, dsl]                      # [b, blk(x1,x3,x2,x4), pole, d]
        xs = np.ascontiguousarray(
            t.transpose(2, 1, 0, 3).reshape(KC, 128, 2, 2, Bx, dsh))
        in_maps.append({
            "rr": rr_s, "theta": th_s, "xs": xs,
            "x0": np.ascontiguousarray(x16[:, 0, dsl]),
        })
    kw = {}
    if trace:
        kw = {"trace": True, "trace_kwargs": trace_kwargs or {}}
    res = bass_utils.run_bass_kernel_spmd(nc, in_maps,
                                          core_ids=list(range(N_CORES)), **kw)
    out = np.empty((Bx, T, D), dtype=np.float32)
    for core in range(N_CORES):
        buf = res.results[core]["out"]            # [dh, par, jb, j, b, d]
        for dh in range(dsh // 128):
            dcols = slice(core * dsh + dh * 128, core * dsh + (dh + 1) * 128)
            for par in range(2):
                for jb in range(4):
                    tsl = slice(256 * jb + par, 256 * jb + 256, 2)
                    out[:, tsl, dcols] = np.moveaxis(
                        buf[dh, par, jb], 1, 0).astype(np.float32)
    if trace:
        return out, res
    return out


# revision 5
# speedup vs baseline: 1.2323x; 1.0477x over previous
"""Trainium2 Bass kernel for nn_Decoder_481036337511 — v2.

out[b,t,d] = sum_k dic[t,k] x[b,k,d];  dic = normalized real dictionary
[T=1024, 1+4*4096] built on-device from rr/theta; x is [4, 16385, 2048].

Strategy (8 cores data-parallel on D, 256 cols each):
  - Parity trick: dic blocks [ones, A, S*A, B, S*B] with S=diag((-1)^t);
    out[even] = A@U + B@W + ones-term, out[odd] = A@V + B@Z with
    U=x1+x2, V=x1-x2, W=x3+x4, Z=x3-x4 (combines on DVE/Pool, fp16).
  - Truncation: poles sorted by r DESC on host (contraction order is
    free). After unit-normalization a pole's column decays like r^t, so
    chunk c of 128 sorted poles only needs t < tcut_c with
    tcut = ln(1/delta)/ln(1/rmax); only the first chunk needs full T.
    Exact tail error computed host-side is ~1e-5 for delta=1e-3.
  - Orientation: dict chunks are the matmul STATIONARY ([128 poles,
    <=128 j] with parity-interleaved stride-2 cols), x-combines are the
    MOVING ([128, 4b x 128d] = 512 cols per LDWEIGHTS). PSUM out is
    [j, b, d]; the ones-column term is folded in as a 1-partition
    stationary matmul (0.015625 * 2 = 1/sqrt(1024)).
  - Dict built on-device in t-interleaved layout: r^t via
    tensor_tensor_scan (geometric recurrence, initial = 1/G from the
    closed-form norms), angles via iota*theta/2pi with the fp32
    big-constant round trick, sin/cos via ACT Sin table. Build work is
    batched over uniform-tcut chunk groups with stride-0 broadcast APs.
  - x shipped fp16 in matmul-ready layout [chunk, pole, blk, b, d]
    (halves HBM traffic; host does layout/cast only, no arithmetic).
    Output written as fp16 [dh, par, jb, j, b, d] tiles, host casts to
    fp32 and interleaves parities.
"""

import numpy as np
from contextlib import ExitStack

import concourse.bass as bass
import concourse.bacc as bacc
import concourse.mybir as mybir
from concourse import tile
from concourse import bass_utils

F32 = mybir.dt.float32
F16 = mybir.dt.float16
I32 = mybir.dt.int32
AF = mybir.ActivationFunctionType
OP = mybir.AluOpType

N_CORES = 8
PI = float(np.pi)
TWO_PI = float(2 * np.pi)
RSQRT_MAGIC = 0x5F3759DF
RND_BIG = 12582912.0  # 2^23 + 2^22: (q + BIG) - BIG == round(q), |q| < 2^22
NPOLE = 4096
KC = 32
B = 4
DSH = 256
T = 1024
DELTA = 1e-3  # truncation tail target (achieved tail err ~1e-5)
JGRAN = 64
L_RETAIN = 4  # max long chunks whose combines stay resident


def build_kernel_nc(jcuts, newton=1):
    """Emit the per-core Bass program for the given per-chunk j-cuts."""
    jcuts = list(jcuts)
    tcuts = [2 * j for j in jcuts]
    offs = np.concatenate([[0], np.cumsum(tcuts)]).astype(int)
    SUMT = int(offs[-1])
    maxjb = max(int(np.ceil(j / 128)) for j in jcuts)
    njb = 4  # T/256: output always covers 4 j-blocks
    DH = DSH // 128

    # build blocks: runs of equal tcut, split so Ng*tcut <= 1024
    blocks = []
    c = 0
    while c < KC:
        c1 = c
        while c1 < KC and tcuts[c1] == tcuts[c]:
            c1 += 1
        step = max(1, 1024 // tcuts[c])
        for s in range(c, c1, step):
            blocks.append((s, min(c1, s + step)))
        c = c1

    nc = bacc.Bacc("TRN2", target_bir_lowering=False, debug=False)

    rr_d = nc.dram_tensor("rr", [NPOLE], F32, kind="ExternalInput")
    th_d = nc.dram_tensor("theta", [NPOLE], F32, kind="ExternalInput")
    # pair-grouped blocks: [:, :, 0] = [x1|x3], [:, :, 1] = [x2|x4]
    xs_d = nc.dram_tensor("xs", [KC, 128, 2, 2, B, DSH], F16,
                          kind="ExternalInput")
    x0_d = nc.dram_tensor("x0", [B, DSH], F16, kind="ExternalInput")
    out_d = nc.dram_tensor("out", [DH, 2, njb, 128, B, 128], F16,
                           kind="ExternalOutput")

    with tile.TileContext(nc) as tc, ExitStack() as ctx:
        const = ctx.enter_context(tc.tile_pool(name="const", bufs=1))
        cfp = ctx.enter_context(tc.tile_pool(name="cfp", bufs=1))
        dictp = ctx.enter_context(tc.tile_pool(name="dictp", bufs=1))
        bwork = ctx.enter_context(tc.tile_pool(name="bwork", bufs=2))
        xp = ctx.enter_context(tc.tile_pool(name="xp", bufs=6))
        uvp = ctx.enter_context(tc.tile_pool(name="uvp", bufs=4))
        keepp = ctx.enter_context(tc.tile_pool(name="keepp", bufs=1))
        outp = ctx.enter_context(tc.tile_pool(name="outp", bufs=4))
        psp = ctx.enter_context(
            tc.tile_pool(name="ps", bufs=1, space=bass.MemorySpace.PSUM))
        psxp = ctx.enter_context(
            tc.tile_pool(name="psx", bufs=1, space=bass.MemorySpace.PSUM))

        # ---- tiny consts -------------------------------------------------
        rr_t = const.tile([128, KC], F32, tag="rr")
        th_t = const.tile([128, KC], F32, tag="th")
        nc.sync.dma_start(rr_t[:], rr_d[:].rearrange("(kc p) -> p kc", p=128))
        nc.sync.dma_start(th_t[:], th_d[:].rearrange("(kc p) -> p kc", p=128))
        x0_t = const.tile([1, B, DSH], F16, tag="x0")
        nc.sync.dma_start(x0_t[:], x0_d[:])

        hpi = const.tile([128, 1], F32, tag="hpi")
        nc.vector.memset(hpi[:], PI / 2)
        bigp = const.tile([128, 1], F32, tag="bigp")
        nc.vector.memset(bigp[:], RND_BIG)
        bigm = const.tile([128, 1], F32, tag="bigm")
        nc.vector.memset(bigm[:], -RND_BIG)
        ones_h = const.tile([1, 128], F16, tag="ones")
        nc.vector.memset(ones_h[:], 0.015625)  # half of 1/sqrt(T)
        th2p = const.tile([128, KC], F32, tag="th2p")
        nc.vector.tensor_scalar_mul(th2p[:], th_t[:], 1.0 / TWO_PI)

        # ---- closed-form column norms (same math as v1) ------------------
        sinth = const.tile([128, KC], F32, tag="sinth")
        nc.scalar.activation(sinth[:], th_t[:], AF.Sin, bias=0.0, scale=1.0)
        costh = const.tile([128, KC], F32, tag="costh")
        nc.scalar.activation(costh[:], th_t[:], AF.Sin, bias=hpi[:], scale=1.0)
        rs = const.tile([128, KC], F32, tag="rs")
        nc.vector.tensor_mul(rs[:], sinth[:], rr_t[:])
        maskB = const.tile([128, KC], F32, tag="maskB")
        nc.vector.tensor_scalar(maskB[:], rs[:], 0.0, None, op0=OP.is_gt)
        th2p = const.tile([128, KC], F32, tag="th2p")
        nc.vector.tensor_scalar_mul(th2p[:], th_t[:], 1.0 / TWO_PI)

        r2j = const.tile([128, 10, KC], F32, tag="r2j")
        nc.vector.tensor_mul(r2j[:, 0], rr_t[:], rr_t[:])
        for j in range(1, 10):
            nc.vector.tensor_mul(r2j[:, j], r2j[:, j - 1], r2j[:, j - 1])

        def cf(name):
            return cfp.tile([128, KC], F32, tag=name, name=name)

        R_ = r2j[:, 0]
        rt = cf("rt")
        nc.vector.tensor_mul(rt[:], r2j[:, 9], r2j[:, 9])
        omr = cf("omr")
        nc.vector.tensor_scalar(omr[:], rr_t[:], -1.0, 1.0, op0=OP.mult,
                                op1=OP.add)
        opr = cf("opr")
        nc.vector.tensor_scalar(opr[:], rr_t[:], 1.0, None, op0=OP.add)
        omR = cf("omR")
        nc.vector.tensor_mul(omR[:], omr[:], opr[:])
        ssq2 = cf("ssq2")
        nc.vector.scalar_tensor_tensor(ssq2[:], sinth[:], 2.0, sinth[:],
                                       op0=OP.mult, op1=OP.mult)
        s2t = cf("s2t")
        nc.vector.scalar_tensor_tensor(s2t[:], sinth[:], 2.0, costh[:],
                                       op0=OP.mult, op1=OP.mult)
        zim = cf("zim")
        nc.vector.tensor_mul(zim[:], R_, s2t[:])
        rmz = cf("rmz")
        nc.vector.tensor_mul(rmz[:], R_, ssq2[:])
        a1r = cf("a1r")
        nc.vector.tensor_add(a1r[:], omR[:], rmz[:])
        qq = cf("qq")
        nc.vector.tensor_scalar(qq[:], th2p[:], 2048.0, None, op0=OP.mult)
        qqr = cf("qqr")
        nc.vector.tensor_scalar(qqr[:], qq[:], RND_BIG, -RND_BIG, op0=OP.add,
                                op1=OP.add)
        dd = cf("dd")
        nc.vector.scalar_tensor_tensor(dd[:], qqr[:], -1.0, qq[:],
                                       op0=OP.mult, op1=OP.add)
        adt = cf("adt")
        nc.vector.tensor_scalar(adt[:].bitcast(I32), dd[:].bitcast(I32),
                                0x7FFFFFFF, None, op0=OP.bitwise_and)
        sT = cf("sT")
        nc.scalar.activation(sT[:], dd[:], AF.Sin, bias=0.0, scale=TWO_PI)
        cT = cf("cT")
        nc.scalar.activation(cT[:], adt[:], AF.Sin, bias=hpi[:], scale=-TWO_PI)
        zTr = cf("zTr")
        nc.vector.tensor_mul(zTr[:], rt[:], cT[:])
        zTi = cf("zTi")
        nc.vector.tensor_mul(zTi[:], rt[:], sT[:])
        omrt = cf("omrt")
        nc.vector.tensor_scalar(omrt[:], rt[:], -1.0, 1.0, op0=OP.mult,
                                op1=OP.add)
        rrec = cf("rrec")
        nc.vector.reciprocal(rrec[:], omR[:])
        s0_ = cf("s0_")
        nc.vector.tensor_mul(s0_[:], omrt[:], rrec[:])
        xx = cf("xx")
        nc.vector.tensor_scalar(xx[:], zTr[:], -1.0, 1.0, op0=OP.mult,
                                op1=OP.add)
        n1 = cf("n1")
        nc.vector.tensor_mul(n1[:], xx[:], a1r[:])
        n2 = cf("n2")
        nc.vector.tensor_mul(n2[:], zTi[:], zim[:])
        num = cf("num")
        nc.vector.tensor_add(num[:], n1[:], n2[:])
        dn1 = cf("dn1")
        nc.vector.tensor_mul(dn1[:], a1r[:], a1r[:])
        dn2 = cf("dn2")
        nc.vector.tensor_mul(dn2[:], zim[:], zim[:])
        den = cf("den")
        nc.vector.tensor_add(den[:], dn1[:], dn2[:])
        rden = cf("rden")
        nc.vector.reciprocal(rden[:], den[:])
        reC = cf("reC")
        nc.vector.tensor_mul(reC[:], num[:], rden[:])
        g2t = const.tile([128, 2, KC], F32, tag="g2t")
        nc.vector.tensor_add(g2t[:, 0], s0_[:], reC[:])
        nc.vector.tensor_scalar_mul(g2t[:, 0], g2t[:, 0], 0.5)
        nr1 = cf("nr1")
        nc.vector.tensor_mul(nr1[:], rt[:], a1r[:])
        nr2 = cf("nr2")
        nc.vector.tensor_mul(nr2[:], zTr[:], omR[:])
        nre = cf("nre")
        nc.vector.tensor_sub(nre[:], rmz[:], nr1[:])
        nc.vector.tensor_add(nre[:], nre[:], nr2[:])
        ni1 = cf("ni1")
        nc.vector.tensor_mul(ni1[:], zim[:], omrt[:])
        ni2 = cf("ni2")
        nc.vector.tensor_mul(ni2[:], zTi[:], omR[:])
        nim = cf("nim")
        nc.vector.tensor_sub(nim[:], ni2[:], ni1[:])
        dre = cf("dre")
        nc.vector.tensor_mul(dre[:], omR[:], a1r[:])
        dimp = cf("dimp")
        nc.vector.tensor_mul(dimp[:], omR[:], zim[:])
        m1_ = cf("m1_")
        nc.vector.tensor_mul(m1_[:], nre[:], dre[:])
        m2_ = cf("m2_")
        nc.vector.tensor_mul(m2_[:], nim[:], dimp[:])
        mnum = cf("mnum")
        nc.vector.tensor_sub(mnum[:], m1_[:], m2_[:])
        e1_ = cf("e1_")
        nc.vector.tensor_mul(e1_[:], dre[:], dre[:])
        e2_ = cf("e2_")
        nc.vector.tensor_mul(e2_[:], dimp[:], dimp[:])
        eden = cf("eden")
        nc.vector.tensor_add(eden[:], e1_[:], e2_[:])
        rede = cf("rede")
        nc.vector.reciprocal(rede[:], eden[:])
        nc.vector.tensor_mul(g2t[:, 1], mnum[:], rede[:])
        nc.vector.tensor_scalar_mul(g2t[:, 1], g2t[:, 1], 0.5)
        gcl = const.tile([128, 2, KC], F32, tag="gcl")
        nc.vector.tensor_scalar_max(gcl[:], g2t[:], 1e-30)
        y0i = const.tile([128, 2, KC], I32, tag="y0i")
        nc.vector.tensor_scalar(y0i[:], gcl[:].bitcast(I32), 1, None,
                                op0=OP.arith_shift_right)
        invgt = const.tile([128, 2, KC], F32, tag="invgt")
        y_t = invgt
        nc.vector.tensor_scalar(y_t[:].bitcast(I32), y0i[:], -1, RSQRT_MAGIC,
                                op0=OP.mult, op1=OP.add)
        yy = const.tile([128, 2, KC], F32, tag="yy")
        ff = const.tile([128, 2, KC], F32, tag="ff")
        for it in range(newton + 1):
            nc.vector.tensor_mul(yy[:], y_t[:], y_t[:])
            nc.vector.tensor_mul(yy[:], yy[:], gcl[:])
            nc.vector.tensor_scalar(ff[:], yy[:], -0.5, 1.5, op0=OP.mult,
                                    op1=OP.add)
            nc.vector.tensor_mul(y_t[:], y_t[:], ff[:])
        invgA = invgt[:, 0]
        # sclB = 2*invgB*mask (the 2 from sin(2pi d) = 2 sin(pi d) cos(pi d))
        sclB = const.tile([128, KC], F32, tag="sclB")
        nc.vector.scalar_tensor_tensor(sclB[:], invgt[:, 1], 2.0, maskB[:],
                                       op0=OP.mult, op1=OP.mult)

        # ---- t index (iota), shared by all build blocks ------------------
        tmax = max(tcuts)
        tf_i = const.tile([128, tmax], I32, tag="tfi")
        nc.gpsimd.iota(tf_i[:], pattern=[[1, tmax]], base=0,
                       channel_multiplier=0)
        tf32 = const.tile([128, tmax], F32, tag="tf32")
        nc.gpsimd.tensor_copy(tf32[:], tf_i[:])

        # ---- dictionary build (batched per uniform-tcut block) -----------
        adict = dictp.tile([128, SUMT], F16, tag="adict")
        bdict = dictp.tile([128, SUMT], F16, tag="bdict")
        pwA = dictp.tile([128, SUMT], F16, tag="pwA")

        def build_block(c0, c1):
            ng = c1 - c0
            tc_ = tcuts[c0]
            o0 = int(offs[c0])
            ncol = ng * tc_
            # pwA = r^t via geometric scan (fp32 state, fp16 out); the 1/G
            # normalization is applied later as a per-chunk ACT scale so the
            # scans do not wait on the closed-form-norm dependency chain
            seg = pwA[:, o0:o0 + ncol].rearrange("p (c t) -> p c t", t=tc_)
            nc.vector.memset(seg[:, :, 0], 1.0)
            for c in range(c0, c1):
                d0 = rr_t[:, c:c + 1].to_broadcast([128, tc_ - 1])
                nc.vector.tensor_tensor_scan(
                    pwA[:, offs[c] + 1:offs[c] + tc_], d0, d0,
                    1.0, op0=OP.mult, op1=OP.bypass)
            # half-angle: d = psi - round(psi); snh=sin(pi d), cnh=cos(pi d)
            psi = bwork.tile([128, ng, tc_], F32, tag="w1", name="psi")
            nc.gpsimd.tensor_tensor(
                psi[:], tf32[:, :tc_].unsqueeze(1).to_broadcast([128, ng, tc_]),
                th2p[:, c0:c1].unsqueeze(2).to_broadcast([128, ng, tc_]),
                op=OP.mult)
            qh = bwork.tile([128, ng, tc_], F32, tag="w2", name="qh")
            nc.scalar.activation(qh[:], psi[:], AF.Identity, bias=bigp[:],
                                 scale=1.0)
            nc.scalar.activation(qh[:], qh[:], AF.Identity, bias=bigm[:],
                                 scale=1.0)
            dt_ = bwork.tile([128, ng, tc_], F32, tag="w3", name="dt")
            nc.gpsimd.tensor_sub(dt_[:], psi[:], qh[:])
            snh = bwork.tile([128, ng, tc_], F16, tag="w4", name="snh")
            nc.scalar.activation(snh[:], dt_[:], AF.Sin, bias=0.0, scale=PI)
            cnh = bwork.tile([128, ng, tc_], F16, tag="w5", name="cnh")
            nc.scalar.activation(cnh[:], dt_[:], AF.Sin, bias=hpi[:],
                                 scale=-PI)
            # cos(2pi d) = 1 - 2 snh^2 ; sin(2pi d) = 2 snh cnh (2 in sclB)
            sq = bwork.tile([128, ng, tc_], F16, tag="w6", name="sq")
            nc.vector.tensor_tensor(sq[:], snh[:], snh[:], op=OP.mult)
            c2 = bwork.tile([128, ng, tc_], F16, tag="w7", name="c2")
            nc.scalar.activation(c2[:], sq[:], AF.Identity, bias=1.0,
                                 scale=-2.0)
            snc = bwork.tile([128, ng, tc_], F16, tag="w8", name="snc")
            nc.vector.tensor_tensor(snc[:], snh[:], cnh[:], op=OP.mult)
            aseg = adict[:, o0:o0 + ncol].rearrange("p (c t) -> p c t", t=tc_)
            bseg = bdict[:, o0:o0 + ncol].rearrange("p (c t) -> p c t", t=tc_)
            pa = pwA[:, o0:o0 + ncol].rearrange("p (c t) -> p c t", t=tc_)
            nc.vector.tensor_tensor(aseg[:], c2[:], pa[:], op=OP.mult)
            nc.vector.tensor_tensor(bseg[:], snc[:], pa[:], op=OP.mult)
            # late normalization: per-chunk 1/G scales (ACT, per-partition AP)
            for c in range(c0, c1):
                asl = adict[:, offs[c]:offs[c] + tc_]
                nc.scalar.activation(asl, asl, AF.Identity, bias=0.0,
                                     scale=invgA[:, c:c + 1])
                bsl = bdict[:, offs[c]:offs[c] + tc_]
                nc.scalar.activation(bsl, bsl, AF.Identity, bias=0.0,
                                     scale=sclB[:, c:c + 1])

        # ---- x load + combines ------------------------------------------
        def load_combine(c, keep=False):
            pool = keepp if keep else uvp
            xt = xp.tile([128, 2, 2, B, DSH], F16, tag="xt", name=f"xt{c}")
            nc.sync.dma_start(xt[:], xs_d[c])
            sfx = f"k{c}" if keep else "s"
            uw = pool.tile([128, 2, B, DSH], F16, tag=f"uw{sfx}", name=f"uw{c}")
            vz = pool.tile([128, 2, B, DSH], F16, tag=f"vz{sfx}", name=f"vz{c}")
            nc.vector.tensor_add(uw[:], xt[:, 0], xt[:, 1])
            nc.vector.tensor_sub(vz[:], xt[:, 0], xt[:, 1])
            return uw[:, 0], vz[:, 0], uw[:, 1], vz[:, 1]

        long_chunks = [c for c in range(KC) if jcuts[c] > 128]
        kept = {}

        built = set()

        def ensure_block(c):
            for (c0, c1) in blocks:
                if c0 <= c < c1 and (c0, c1) not in built:
                    built.add((c0, c1))
                    build_block(c0, c1)

        ensure_block(0)
        if KC > 1:
            ensure_block(1)
        for c in long_chunks[:L_RETAIN]:
            kept[c] = load_combine(c, keep=True)

        def stat_ap(dic, c, jb, par, h):
            seg = dic[:, offs[c]:offs[c] + tcuts[c]].rearrange(
                "p (j two) -> p j two", two=2)
            return seg[:, 128 * jb:128 * jb + h, par]

        def chunk_mms(c, jb, uvwz, psums, first, last):
            u, v, w, z = uvwz
            h = min(128, jcuts[c] - 128 * jb)
            for dh in range(DH):
                dsl = slice(dh * 128, (dh + 1) * 128)
                for par, (mA, mB) in enumerate(((u, w), (v, z))):
                    ps = psums[(dh, par)]
                    if first:
                        nc.tensor.matmul(ps[:], ones_h[:], x0_t[:, :, dsl],
                                         start=True, stop=False)
                    nc.tensor.matmul(ps[0:h], stat_ap(adict, c, jb, par, h),
                                     mA[:, :, dsl], start=False, stop=False)
                    nc.tensor.matmul(ps[0:h], stat_ap(bdict, c, jb, par, h),
                                     mB[:, :, dsl], start=False, stop=False)
                    if last:
                        nc.tensor.matmul(ps[:], ones_h[:], x0_t[:, :, dsl],
                                         start=False, stop=True)

        def open_psums(pool):
            return {(dh, par): pool.tile([128, B, 128], F32,
                                         tag=f"ps{dh}{par}",
                                         name=f"ps{dh}{par}")
                    for dh in range(DH) for par in range(2)}

        def run_pass(jb, chunks, pool=None, interleave_build=False,
                     after_first=None):
            psums = open_psums(pool or psp)
            if not chunks:
                # ones-term only (general-input safety; never hit when
                # some chunk reaches this j-block)
                for dh in range(DH):
                    dsl = slice(dh * 128, (dh + 1) * 128)
                    for par in range(2):
                        ps = psums[(dh, par)]
                        nc.tensor.matmul(ps[:], ones_h[:], x0_t[:, :, dsl],
                                         start=True, stop=False)
                        nc.tensor.matmul(ps[:], ones_h[:], x0_t[:, :, dsl],
                                         start=False, stop=True)
            else:
                for i, c in enumerate(chunks):
                    if interleave_build:
                        ensure_block(c)
                        ensure_block(min(c + 8, KC - 1), pre_only=True)
                    uvwz = kept.get(c)
                    if uvwz is None:
                        uvwz = load_combine(c)
                    chunk_mms(c, jb, uvwz, psums, i == 0, i == len(chunks) - 1)
                    if i == 0 and after_first is not None:
                        after_first()
            # drain: ACT copy psum -> sbuf fp16, DMA out on scalar queue
            for (dh, par), ps in psums.items():
                ob = outp.tile([128, B, 128], F16, tag="ob",
                               name=f"ob{jb}{dh}{par}")
                nc.scalar.activation(ob[:], ps[:], AF.Identity, bias=0.0,
                                     scale=1.0)
                nc.scalar.dma_start(out_d[dh, par, jb], ob[:])

        # main jb0 pass streams all chunks (dict blocks built just-in-time
        # so DVE work interleaves with the stream). The extra j-blocks only
        # touch the retained long chunks; they run INSIDE the stream right
        # after chunk 0 (own psum bank set), filling the PE's early wait
        # gap and removing the drain tail.
        def extras():
            for jb in range(1, njb):
                run_pass(jb, [c for c in range(KC) if jcuts[c] > 128 * jb],
                         pool=psxp)

        run_pass(0, list(range(KC)), interleave_build=True,
                 after_first=extras)

    nc.compile()
    return nc


_NC_CACHE = {}


def _get_nc(jcuts):
    key = tuple(jcuts)
    if key not in _NC_CACHE:
        _NC_CACHE[key] = build_kernel_nc(key)
    return _NC_CACHE[key]


def _jcuts(rr_sorted):
    L = float(np.log(1.0 / DELTA))
    jc = []
    for c in range(KC):
        rmax = float(rr_sorted[c * 128])
        neg = max(-np.log(max(rmax, 1e-30)), 1e-12)
        j = L / (2.0 * neg)
        j = int(np.ceil(j / JGRAN)) * JGRAN
        jc.append(int(np.clip(j, JGRAN, 512)))
    return jc


def kernel(rr, theta, x, trace=False, trace_kwargs=None):
    rr = np.ascontiguousarray(np.asarray(rr, dtype=np.float32))
    theta = np.ascontiguousarray(np.asarray(theta, dtype=np.float32))
    x = np.asarray(x, dtype=np.float32)
    Bx, KTOT, D = x.shape
    dsh = D // N_CORES

    perm = np.argsort(-rr, kind="stable")
    rr_s = np.ascontiguousarray(rr[perm])
    th_s = np.ascontiguousarray(theta[perm])
    jcuts = _jcuts(rr_s)
    nc = _get_nc(jcuts)

    x16 = x.astype(np.float16)
    # pair-grouped block order (x1,x3),(x2,x4) so one tensor_tensor makes
    # [U|W] and one makes [V|Z]
    xb = x16[:, 1:, :].reshape(Bx, 4, NPOLE, D)[:, [0, 2, 1, 3], :, :]
    xb = xb[:, :, perm, :]
    in_maps = []
    for core in range(N_CORES):
        dsl = slice(core * dsh, (core + 1) * dsh)
        t = xb[:, :, :, dsl]                      # [b, blk(x1,x3,x2,x4), pole, d]
        xs = np.ascontiguousarray(
            t.transpose(2, 1, 0, 3).reshape(KC, 128, 2, 2, Bx, dsh))
        in_maps.append({
            "rr": rr_s, "theta": th_s, "xs": xs,
            "x0": np.ascontiguousarray(x16[:, 0, dsl]),
        })
    kw = {}
    if trace:
        kw = {"trace": True, "trace_kwargs": trace_kwargs or {}}
    res = bass_utils.run_bass_kernel_spmd(nc, in_maps,
                                          core_ids=list(range(N_CORES)), **kw)
    out = np.empty((Bx, T, D), dtype=np.float32)
    for core in range(N_CORES):
        buf = res.results[core]["out"]            # [dh, par, jb, j, b, d]
        for dh in range(dsh // 128):
            dcols = slice(core * dsh + dh * 128, core * dsh + (dh + 1) * 128)
            for par in range(2):
                for jb in range(4):
                    tsl = slice(256 * jb + par, 256 * jb + 256, 2)
                    out[:, tsl, dcols] = np.moveaxis(
                        buf[dh, par, jb], 1, 0).astype(np.float32)
    if trace:
        return out, res
    return out
